# revision 12
# baseline (speedup 1.0000x reference)
"""Trainium2 Bass kernel for nn_MultiHeadAttention (B=2, S=2048, D=1024, H=16).

Sharding: 8 cores = (batch b in {0,1}) x (head-group g in {0..3}, 4 heads each).
Tensor-parallel on W_Q/W_K/W_V columns and W_O rows; batch-parallel on top.

Per core:
  - project q,k,v for its 4 heads (fp32r matmuls at full PE rate)
  - causal attention: scores computed in BOTH orientations:
      [q,k] -> exp (+row sums via ACT accum_out) -> normalize -> attn_weights out
      [k,q] -> exp -> attn@V matmul (no transposes needed anywhere)
  - output projection -> partial yT (transposed) shipped to host
Host: assembles attn_weights, sums the 4 per-batch partials, adds residual,
applies LayerNorm, returns (out, attn_weights) like the reference.
"""

import sys

sys.path.insert(0, "/opt/trn_rl_repo")

import numpy as np

import concourse.bass as bass
from concourse import bacc, mybir
from concourse.bass_utils import run_bass_kernel_spmd
from concourse.tile import TileContext
from concourse.masks import make_identity

B = 2
S = 2048
D_MODEL = 1024
NUM_HEADS = 16
D_K = 64
LN_EPS = 1e-5
NEG = -1.0e6  # additive mask; exp underflows to exact 0.0 in f32

N_CORES = 8
GROUPS = 4          # head groups (4 heads each)
HPC = 4             # heads per core
DCORE = HPC * D_K   # 256 head-dims per core

F32 = mybir.dt.float32
F32R = mybir.dt.float32r

NQC = S // 128      # 16 q-chunks of 128
NQ4 = S // 512      # 4 q-chunks of 512
ND8 = D_MODEL // 128  # 8 model-dim chunks


def _build(mode):
    """mode: 'causal' (tril mask), 'full' (all-ones mask), 'masked' (arbitrary)."""
    assert mode in ("causal", "full", "masked")
    nc = bacc.Bacc(None, target_bir_lowering=False)

    xqT = nc.declare_dram_parameter("xqT", [D_MODEL, S], F32, isOutput=False)
    xkT = nc.declare_dram_parameter("xkT", [D_MODEL, S], F32, isOutput=False)
    xvT = nc.declare_dram_parameter("xvT", [D_MODEL, S], F32, isOutput=False)
    wqT = nc.declare_dram_parameter("wqT", [D_MODEL, DCORE], F32, isOutput=False)
    wkT = nc.declare_dram_parameter("wkT", [D_MODEL, DCORE], F32, isOutput=False)
    wvT = nc.declare_dram_parameter("wvT", [D_MODEL, DCORE], F32, isOutput=False)
    woT = nc.declare_dram_parameter("woT", [2, 128, D_MODEL], F32, isOutput=False)
    if mode == "causal":
        mq_d = nc.declare_dram_parameter("mask_qk", [4, 128, 512], F32, isOutput=False)
        mt_d = nc.declare_dram_parameter("mask_et", [4, 128, 512], F32, isOutput=False)
    if mode == "masked":
        am_d = nc.declare_dram_parameter("amask", [S, S], F32, isOutput=False)
        amT_d = nc.declare_dram_parameter("amaskT", [S, S], F32, isOutput=False)

    attnw = nc.declare_dram_parameter("attnw", [HPC, S, S], F32, isOutput=True)
    yT = nc.declare_dram_parameter("yT", [D_MODEL, S], F32, isOutput=True)

    Exp = mybir.ActivationFunctionType.Exp

    with TileContext(nc) as tc:
        from contextlib import ExitStack

        with ExitStack() as ctx:
            consts = ctx.enter_context(tc.tile_pool(name="consts", bufs=1))
            xin = ctx.enter_context(tc.tile_pool(name="xin", bufs=1))
            resid = ctx.enter_context(tc.tile_pool(name="resid", bufs=1))
            eqk = ctx.enter_context(tc.tile_pool(name="eqk", bufs=2))
            etp = ctx.enter_context(tc.tile_pool(name="etp", bufs=3))
            small = ctx.enter_context(tc.tile_pool(name="small", bufs=4))
            rbp = ctx.enter_context(tc.tile_pool(name="rbp", bufs=2))
            ystg = ctx.enter_context(tc.tile_pool(name="ystg", bufs=2))
            if mode == "masked":
                amp = ctx.enter_context(tc.tile_pool(name="amp", bufs=3))
                amtp = ctx.enter_context(tc.tile_pool(name="amtp", bufs=3))
            dramp = ctx.enter_context(tc.tile_pool(name="dramp", bufs=1, space="DRAM"))
            ps_proj = ctx.enter_context(tc.tile_pool(name="ps_proj", bufs=2, space="PSUM"))
            ps_s = ctx.enter_context(tc.tile_pool(name="ps_s", bufs=2, space="PSUM"))
            ps_t = ctx.enter_context(tc.tile_pool(name="ps_t", bufs=2, space="PSUM"))
            ps_o = ctx.enter_context(tc.tile_pool(name="ps_o", bufs=1, space="PSUM"))

            # ---- constants ----
            wq_s = consts.tile([128, ND8, DCORE], F32R, tag="wq")
            wk_s = consts.tile([128, ND8, DCORE], F32R, tag="wk")
            wv_s = consts.tile([128, ND8, DCORE], F32R, tag="wv")
            # dram [D_MODEL, DCORE] -> sbuf [128, ND8, DCORE]
            for wt, wd in ((wq_s, wqT), (wk_s, wkT), (wv_s, wvT)):
                nc.gpsimd.dma_start(
                    out=wt, in_=wd.rearrange("(n p) d -> p n d", p=128)
                )
            wo_s = consts.tile([128, 2, D_MODEL], F32R, tag="wo")
            nc.gpsimd.dma_start(out=wo_s, in_=woT.rearrange("c p d -> p c d"))
            if mode == "causal":
                mq_s = consts.tile([128, 4, 512], F32, tag="mq")
                nc.sync.dma_start(out=mq_s, in_=mq_d.rearrange("m p c -> p m c"))
                mt_s = consts.tile([128, 4, 512], F32, tag="mt")
                nc.sync.dma_start(out=mt_s, in_=mt_d.rearrange("m p c -> p m c"))
            ident = consts.tile([128, 128], F32, tag="ident")
            make_identity(nc, ident)

            # ---- residents ----
            qT = resid.tile([128, 2, S], F32R, tag="qT")   # part=64*(h%2)+dk, free=(pair, s)
            kT = resid.tile([128, 2, S], F32R, tag="kT")
            vv = resid.tile([128, NQC, DCORE], F32R, tag="v")  # part=k%128, free=(kchunk, dv)
            oT = resid.tile([128, 2, S], F32R, tag="oT")
            r_all = resid.tile([128, HPC, NQC], F32, tag="r_all")
            rT = resid.tile([16, HPC, 128], F32, tag="rT")
            # DRAM scratch holding 1/rowsum in [head, qchunk, qpos] layout so a
            # replicating DMA can build the [d, q] broadcast tiles below.
            rT_dram = dramp.tile([HPC, NQC, 128], F32, tag="rT_dram")

            # ---- phase 1: projections (s-chunks of 256) ----
            for sc in range(S // 256):
                s0 = sc * 256
                xq_t = xin.tile([128, ND8, 256], F32R, tag="xq")
                xk_t = xin.tile([128, ND8, 256], F32R, tag="xk")
                xv_t = xin.tile([128, ND8, 256], F32R, tag="xv")
                for xt, xd in ((xq_t, xqT), (xk_t, xkT), (xv_t, xvT)):
                    nc.gpsimd.dma_start(
                        out=xt,
                        in_=xd[:, s0:s0 + 256].rearrange("(n p) s -> p n s", p=128),
                    )
                for w_s, x_t, dst in ((wq_s, xq_t, qT), (wk_s, xk_t, kT)):
                    for dq in range(2):
                        pt = ps_proj.tile([128, 512], F32, tag="pp")
                        for d8 in range(ND8):
                            nc.tensor.matmul(
                                pt[:, 0:256],
                                w_s[:, d8, dq * 128:(dq + 1) * 128],
                                x_t[:, d8, :],
                                start=(d8 == 0),
                                stop=(d8 == ND8 - 1),
                            )
                        nc.vector.tensor_copy(dst[:, dq, s0:s0 + 256], pt[:, 0:256])
                for sub in range(2):
                    pt = ps_proj.tile([128, 512], F32, tag="pp")
                    for d8 in range(ND8):
                        nc.tensor.matmul(
                            pt[:, 0:256],
                            xv_t[:, d8, sub * 128:(sub + 1) * 128],
                            wv_s[:, d8, :],
                            start=(d8 == 0),
                            stop=(d8 == ND8 - 1),
                        )
                    nc.vector.tensor_copy(vv[:, 2 * sc + sub, :], pt[:, 0:256])

            # ---- phase 2: attention, per head-pair ----
            for pair in range(2):
                heads = (2 * pair, 2 * pair + 1)
                # --- [q,k] side: attn_weights + row sums ---
                for i in range(NQC):
                    nJ = (i // 4 + 1) if mode == "causal" else 4
                    diagJ = i // 4 if mode == "causal" else -1
                    e_t = {}
                    acc = {}
                    for h in heads:
                        e_t[h] = eqk.tile([128, S], F32, tag=f"e{h % 2}", name=f"e_{h % 2}")
                        acc[h] = small.tile([128, 4], F32, tag=f"acc{h % 2}", name=f"acc_{h % 2}")
                    for J in range(nJ):
                        am_t = None
                        if mode == "masked":
                            am_t = amp.tile([128, 512], F32, tag="am")
                            nc.sync.dma_start(
                                out=am_t,
                                in_=am_d[i * 128:(i + 1) * 128, J * 512:(J + 1) * 512],
                            )
                        for h in heads:
                            hl = h % 2
                            ps = ps_s.tile([128, 512], F32, tag="ps")
                            nc.tensor.matmul(
                                ps,
                                qT[64 * hl:64 * (hl + 1), pair, i * 128:(i + 1) * 128],
                                kT[64 * hl:64 * (hl + 1), pair, J * 512:(J + 1) * 512],
                                start=True,
                                stop=True,
                                tile_position=(64 * hl, 0),
                            )
                            if J == diagJ:
                                nc.vector.tensor_add(ps, ps, mq_s[:, i % 4, :])
                            elif am_t is not None:
                                nc.vector.tensor_add(ps, ps, am_t)
                            nc.scalar.activation(
                                e_t[h][:, J * 512:(J + 1) * 512], ps, Exp,
                                accum_out=acc[h][:, J:J + 1],
                            )
                    for h in heads:
                        ssum = small.tile([128, 1], F32, tag="ssum")
                        nc.vector.reduce_sum(ssum, acc[h][:, 0:nJ], axis=mybir.AxisListType.X)
                        nc.vector.reciprocal(r_all[:, h, i:i + 1], ssum)
                        nc.vector.tensor_scalar_mul(
                            e_t[h][:, 0:512 * nJ], e_t[h][:, 0:512 * nJ],
                            r_all[:, h, i:i + 1],
                        )
                        nc.sync.dma_start(
                            out=attnw[2 * pair + h % 2, i * 128:(i + 1) * 128, 0:512 * nJ],
                            in_=e_t[h][:, 0:512 * nJ],
                        )
                # --- transpose reciprocal sums: r_all [128, 16] -> rT [16, 128] ---
                for h in heads:
                    pr = ps_proj.tile([128, 512], F32, tag="pp")
                    nc.tensor.transpose(pr[0:16, 0:128], r_all[:, h, :], ident)
                    nc.vector.tensor_copy(rT[:, h, :], pr[0:16, 0:128])
                    nc.sync.dma_start(out=rT_dram[h, :, :], in_=rT[:, h, :])
                # --- [k,q] side: exp(scores^T) @ v -> oT ---
                for q4 in range(NQ4):
                    nj = (4 * q4 + 4) if mode == "causal" else NQC
                    # fp32r matmul output must start at psum partition 0, so each
                    # head accumulates in its own [64, 512] tile; the odd head is
                    # shifted to partitions 64..127 after normalization by DMA.
                    ot_ps = {
                        heads[0]: ps_o.tile([64, 512], F32, tag="po0", name="ot_e"),
                        heads[1]: ps_o.tile([64, 512], F32, tag="po1", name="ot_o"),
                    }
                    for j in range(nj):
                        amt_t = None
                        if mode == "masked":
                            amt_t = amtp.tile([128, 512], F32, tag="amt")
                            nc.sync.dma_start(
                                out=amt_t,
                                in_=amT_d[j * 128:(j + 1) * 128, q4 * 512:(q4 + 1) * 512],
                            )
                        for h in heads:
                            hl = h % 2
                            pst = ps_t.tile([128, 512], F32, tag="pt")
                            nc.tensor.matmul(
                                pst,
                                kT[64 * hl:64 * (hl + 1), pair, j * 128:(j + 1) * 128],
                                qT[64 * hl:64 * (hl + 1), pair, q4 * 512:(q4 + 1) * 512],
                                start=True,
                                stop=True,
                                tile_position=(64 * hl, 0),
                            )
                            if mode == "causal" and j >= 4 * q4:
                                nc.vector.tensor_add(pst, pst, mt_s[:, j - 4 * q4, :])
                            elif amt_t is not None:
                                nc.vector.tensor_add(pst, pst, amt_t)
                            et_t = etp.tile([128, 512], F32R, tag=f"et{hl}", name=f"et_{hl}")
                            nc.scalar.activation(et_t, pst, Exp)
                            nc.tensor.matmul(
                                ot_ps[h],
                                vv[:, j, h * 64:(h + 1) * 64],
                                et_t,
                                start=(j == 0),
                                stop=(j == nj - 1),
                            )
                    # normalize columns by 1/rowsum and store to oT (f32r).
                    # rb_h[d, 128*c + p] = rT_dram[h, 4*q4 + c, p]  (d broadcast)
                    qsl = slice(q4 * 512, (q4 + 1) * 512)
                    for h in heads:
                        hl = h % 2
                        rb = rbp.tile([64, 512], F32, tag=f"rb{hl}", name=f"rb_{hl}")
                        src = bass.AP(
                            tensor=rT_dram.tensor,
                            offset=rT_dram.offset + h * NQC * 128 + (4 * q4) * 128,
                            ap=[[0, 64], [128, 4], [1, 128]],
                        )
                        nc.sync.dma_start(out=rb, in_=src)
                        if hl == 0:
                            nc.vector.tensor_mul(oT[0:64, pair, qsl], ot_ps[h], rb)
                        else:
                            ostg = rbp.tile([64, 512], F32R, tag="ostg", name="ostg")
                            nc.vector.tensor_mul(ostg, ot_ps[h], rb)
                            nc.sync.dma_start(out=oT[64:128, pair, qsl], in_=ostg)

            # ---- phase 3: output projection -> yT ----
            for q4 in range(NQ4):
                for dm in range(ND8):
                    yp = ps_proj.tile([128, 512], F32, tag="pp")
                    for c2 in range(2):
                        nc.tensor.matmul(
                            yp,
                            wo_s[:, c2, dm * 128:(dm + 1) * 128],
                            oT[:, c2, q4 * 512:(q4 + 1) * 512],
                            start=(c2 == 0),
                            stop=(c2 == 1),
                        )
                    ys = ystg.tile([128, 512], F32, tag="ys")
                    nc.vector.tensor_copy(ys, yp)
                    nc.sync.dma_start(
                        out=yT[dm * 128:(dm + 1) * 128, q4 * 512:(q4 + 1) * 512],
                        in_=ys,
                    )

    nc.compile()
    return nc


_PROGRAMS = {}


def _get_program(mode):
    if mode not in _PROGRAMS:
        _PROGRAMS[mode] = _build(mode)
    return _PROGRAMS[mode]


def _causal_mask_tiles():
    p = np.arange(128, dtype=np.int64)[:, None]
    c = np.arange(512, dtype=np.int64)[None, :]
    mq = np.zeros((4, 128, 512), np.float32)
    mt = np.zeros((4, 128, 512), np.float32)
    for m in range(4):
        mq[m] = np.where(c <= 128 * m + p, 0.0, NEG)
        mt[m] = np.where(c >= 128 * m + p, 0.0, NEG)
    return mq, mt


def _detect_mode(mask):
    tril = np.tril(np.ones((S, S), np.int32))
    if all(np.array_equal(np.asarray(mask[b]), tril) for b in range(B)):
        return "causal"
    if np.all(np.asarray(mask) != 0):
        return "full"
    return "masked"


def make_in_maps(Q, K, V, mask, W_Q, W_K, W_V, W_O, mode):
    mq, mt = _causal_mask_tiles()
    in_maps = []
    for c in range(N_CORES):
        b, g = c // 4, c % 4
        hs = slice(DCORE * g, DCORE * (g + 1))
        m = {
            "xqT": np.ascontiguousarray(Q[b].T),
            "xkT": np.ascontiguousarray(K[b].T),
            "xvT": np.ascontiguousarray(V[b].T),
            "wqT": np.ascontiguousarray((W_Q[hs, :] * 0.125).T),
            "wkT": np.ascontiguousarray(W_K[hs, :].T),
            "wvT": np.ascontiguousarray(W_V[hs, :].T),
            "woT": np.ascontiguousarray(
                np.stack([W_O[:, DCORE * g + 128 * c2: DCORE * g + 128 * (c2 + 1)].T
                          for c2 in range(2)])),
        }
        if mode == "causal":
            m["mask_qk"] = mq
            m["mask_et"] = mt
        if mode == "masked":
            am = np.where(np.asarray(mask[b]) == 0, np.float32(NEG), np.float32(0.0))
            m["amask"] = am
            m["amaskT"] = np.ascontiguousarray(am.T)
        in_maps.append(m)
    return in_maps


def _assemble(results, Q, ln_gamma, ln_beta):
    attn = np.empty((B, NUM_HEADS, S, S), np.float32)
    out = np.empty((B, S, D_MODEL), np.float32)
    for b in range(B):
        y = np.zeros((S, D_MODEL), np.float64)
        for g in range(GROUPS):
            res = results[b * 4 + g]
            attn[b, 4 * g:4 * (g + 1)] = res["attnw"]
            y += res["yT"].T
        y += np.asarray(Q[b], np.float64)
        mu = y.mean(axis=-1, keepdims=True)
        var = ((y - mu) ** 2).mean(axis=-1, keepdims=True)
        o = (y - mu) / np.sqrt(var + LN_EPS)
        o = o * np.asarray(ln_gamma, np.float64) + np.asarray(ln_beta, np.float64)
        out[b] = o.astype(np.float32)
    return out, attn


def kernel(Q, K, V, mask, W_Q, W_K, W_V, W_O, ln_gamma, ln_beta):
    Q = np.asarray(Q, np.float32)
    K = np.asarray(K, np.float32)
    V = np.asarray(V, np.float32)
    W_Q = np.asarray(W_Q, np.float32)
    W_K = np.asarray(W_K, np.float32)
    W_V = np.asarray(W_V, np.float32)
    W_O = np.asarray(W_O, np.float32)
    mode = _detect_mode(mask)
    nc = _get_program(mode)
    in_maps = make_in_maps(Q, K, V, mask, W_Q, W_K, W_V, W_O, mode)
    res = run_bass_kernel_spmd(nc, in_maps, core_ids=list(range(N_CORES)))
    return _assemble(res.results, Q, ln_gamma, ln_beta)


# revision 24
# speedup vs baseline: 3.4909x; 3.4909x over previous
"""Trainium2 Bass kernel for nn_MultiHeadAttention (B=2, S=2048, D=1024, H=16).

Sharding: 8 cores = (batch b in {0,1}) x (head-group g in {0..3}, 4 heads each).
Tensor-parallel on W_Q/W_K/W_V columns and W_O rows; batch-parallel on top.

Per core:
  - project q,k (transposed layout) and v for its 4 heads; fp32r matmuls run
    at full PE rate.
  - attention in [k, q] orientation only: scoresT = k @ qT (causal blocks
    only, additive mask accumulated into PSUM on the PE), exp on ScalarE with
    bf16 output. Each exp tile is both shipped to HBM (unnormalized,
    transposed attention weights) and fed to the attn@V matmul. V carries an
    extra ones-column, so the softmax row sums fall out of the same matmul.
  - o = (e @ v_aug) / sums, output projection -> partial yT.
Host: assembles attention weights (upcast + normalize + transpose), sums the
4 per-batch yT partials, adds residual, applies LayerNorm; returns
(out, attn_weights) like the reference.
"""

import sys

sys.path.insert(0, "/opt/trn_rl_repo")

import numpy as np
import ml_dtypes

import concourse.bass as bass
from concourse import bacc, mybir
from concourse.bass_utils import run_bass_kernel_spmd
from concourse.tile import TileContext

B = 2
S = 2048
D_MODEL = 1024
NUM_HEADS = 16
D_K = 64
LN_EPS = 1e-5
NEG = -1.0e6  # additive mask; exp underflows to exact 0.0

N_CORES = 8
GROUPS = 4          # head groups (4 heads each)
HPC = 4             # heads per core
DCORE = HPC * D_K   # 256 head-dims per core

F32 = mybir.dt.float32
F32R = mybir.dt.float32r
BF16 = mybir.dt.bfloat16

NQC = S // 128        # 16 k-chunks of 128
NQ4 = S // 512        # 4 q-chunks of 512
ND8 = D_MODEL // 128  # 8 model-dim chunks


def _build(mode):
    """mode: 'causal' (tril mask), 'full' (all-ones mask), 'masked' (arbitrary)."""
    assert mode in ("causal", "full", "masked")
    nc = bacc.Bacc(None, target_bir_lowering=False)

    xqT = nc.declare_dram_parameter("xqT", [D_MODEL, S], BF16, isOutput=False)
    xkT = nc.declare_dram_parameter("xkT", [D_MODEL, S], BF16, isOutput=False)
    xvT = nc.declare_dram_parameter("xvT", [D_MODEL, S], BF16, isOutput=False)
    wqT = nc.declare_dram_parameter("wqT", [D_MODEL, DCORE], BF16, isOutput=False)
    wkT = nc.declare_dram_parameter("wkT", [D_MODEL, DCORE], BF16, isOutput=False)
    wvT = nc.declare_dram_parameter("wvT", [D_MODEL, DCORE], BF16, isOutput=False)
    woT = nc.declare_dram_parameter("woT", [2, 128, D_MODEL], F32, isOutput=False)
    if mode == "causal":
        mt_d = nc.declare_dram_parameter("mask_et", [4, 128, 512], F32, isOutput=False)
    if mode == "masked":
        amT_d = nc.declare_dram_parameter("amaskT", [S, S], F32, isOutput=False)

    # transposed, unnormalized exp-scores: attnwT[h, k, q]
    attnwT = nc.declare_dram_parameter("attnwT", [HPC, S, S], BF16, isOutput=True)
    sums_o = nc.declare_dram_parameter("sums", [HPC, S], F32, isOutput=True)
    yT = nc.declare_dram_parameter("yT", [D_MODEL, S], F32, isOutput=True)

    Exp = mybir.ActivationFunctionType.Exp

    with TileContext(nc) as tc:
        from contextlib import ExitStack

        with ExitStack() as ctx:
            consts = ctx.enter_context(tc.tile_pool(name="consts", bufs=1))
            xin = ctx.enter_context(tc.tile_pool(name="xin", bufs=2))
            resid = ctx.enter_context(tc.tile_pool(name="resid", bufs=1))
            etp = ctx.enter_context(tc.tile_pool(name="etp", bufs=4))
            small = ctx.enter_context(tc.tile_pool(name="small", bufs=4))
            rbp = ctx.enter_context(tc.tile_pool(name="rbp", bufs=2))
            ystg = ctx.enter_context(tc.tile_pool(name="ystg", bufs=2))
            if mode == "masked":
                amtp = ctx.enter_context(tc.tile_pool(name="amtp", bufs=3))
            dramp = ctx.enter_context(tc.tile_pool(name="dramp", bufs=1, space="DRAM"))
            ps_proj = ctx.enter_context(tc.tile_pool(name="ps_proj", bufs=1, space="PSUM"))
            ps_sc = ctx.enter_context(tc.tile_pool(name="ps_sc", bufs=3, space="PSUM"))
            ps_o = ctx.enter_context(tc.tile_pool(name="ps_o", bufs=2, space="PSUM"))

            # ---- constants ----
            wq_s = consts.tile([128, ND8, DCORE], BF16, tag="wq")
            wk_s = consts.tile([128, ND8, DCORE], BF16, tag="wk")
            wv_s = consts.tile([128, ND8, DCORE], BF16, tag="wv")
            for wt, wd in ((wq_s, wqT), (wk_s, wkT), (wv_s, wvT)):
                nc.sync.dma_start(
                    out=wt, in_=wd.rearrange("(n p) d -> p n d", p=128)
                )
            wo_s = consts.tile([128, 2, D_MODEL], F32R, tag="wo")
            if mode == "causal":
                mt_s = consts.tile([128, 4, 512], F32, tag="mt")
                nc.sync.dma_start(out=mt_s, in_=mt_d.rearrange("m p c -> p m c"))

            # ---- residents ----
            # qT/kT: part = 64*(h%2) + dk, free = s; one tile per head-pair
            qTp = [resid.tile([128, S], F32R, tag=f"qT{p}", name=f"qT{p}") for p in range(2)]
            kTp = [resid.tile([128, S], F32R, tag=f"kT{p}", name=f"kT{p}") for p in range(2)]
            # v_aug: part = k % 128, free = (kchunk, head, 65); col 64 is ones
            vv = resid.tile([128, NQC, HPC, 65], BF16, tag="v")
            nc.vector.memset(vv[:, :, :, 64:65], 1.0)
            oT = resid.tile([128, 2, S], F32R, tag="oT")
            sums_scr = dramp.tile([HPC, S], F32, tag="sums_scr")

            # ---- phase 1: projections (s-chunks of 256), k/q/v interleaved ----
            for sc in range(S // 256):
                s0 = sc * 256
                xq_t = xin.tile([128, ND8, 256], BF16, tag="xq")
                xk_t = xin.tile([128, ND8, 256], BF16, tag="xk")
                xv_t = xin.tile([128, ND8, 256], BF16, tag="xv")
                for eng, xt, xd in ((nc.gpsimd, xk_t, xkT), (nc.gpsimd, xq_t, xqT),
                                    (nc.sync, xv_t, xvT)):
                    eng.dma_start(
                        out=xt,
                        in_=xd[:, s0:s0 + 256].rearrange("(n p) s -> p n s", p=128),
                    )
                for w_s, x_t, dstl in ((wk_s, xk_t, kTp), (wq_s, xq_t, qTp)):
                    for dq in range(2):
                        pt = ps_proj.tile([128, 512], F32, tag="pp")
                        for d8 in range(ND8):
                            nc.tensor.matmul(
                                pt[:, 0:256],
                                w_s[:, d8, dq * 128:(dq + 1) * 128],
                                x_t[:, d8, :],
                                start=(d8 == 0),
                                stop=(d8 == ND8 - 1),
                            )
                        nc.vector.tensor_copy(dstl[dq][:, s0:s0 + 256], pt[:, 0:256])
                for sub in range(2):
                    pt = ps_proj.tile([128, 512], F32, tag="pp")
                    for d8 in range(ND8):
                        nc.tensor.matmul(
                            pt[:, 0:256],
                            xv_t[:, d8, sub * 128:(sub + 1) * 128],
                            wv_s[:, d8, :],
                            start=(d8 == 0),
                            stop=(d8 == ND8 - 1),
                        )
                    nc.vector.tensor_copy(
                        vv[:, 2 * sc + sub, :, 0:64],
                        pt[:, 0:256].rearrange("p (h d) -> p h d", h=HPC),
                    )

            # wo isn't needed until the first output projection; issuing its
            # (SWDGE cast) load here keeps the startup DMA queues clear.
            nc.gpsimd.dma_start(out=wo_s, in_=woT.rearrange("c p d -> p c d"))

            # ---- phase 2: attention [k,q] + per-q4 output projection ----
            for q4 in (0, 2, 3, 1):
                qsl = slice(q4 * 512, (q4 + 1) * 512)
                for pair in range(2):
                    heads = (2 * pair, 2 * pair + 1)
                    nj = (4 * q4 + 4) if mode == "causal" else NQC
                    # fp32r/bf16 matmul output must start at psum partition 0:
                    # head parity picks the column half; rows 0..63 = o, row 64
                    # = softmax row sums (ones column of v_aug).
                    otp = ps_o.tile([65, 1024], F32, tag="po", name="otp")
                    ot_ps = {heads[0]: otp[:, 0:512], heads[1]: otp[:, 512:1024]}
                    for j in range(nj):
                        amt_t = None
                        if mode == "masked":
                            amt_t = amtp.tile([128, 512], F32, tag="amt")
                            nc.sync.dma_start(
                                out=amt_t,
                                in_=amT_d[j * 128:(j + 1) * 128, q4 * 512:(q4 + 1) * 512],
                            )
                        for h in heads:
                            hl = h % 2
                            masked = (mode == "causal" and j >= 4 * q4) or (amt_t is not None)
                            pst = ps_sc.tile([128, 512], F32, tag="ps", name="pst")
                            nc.tensor.matmul(
                                pst,
                                kTp[pair][64 * hl:64 * (hl + 1), j * 128:(j + 1) * 128],
                                qTp[pair][64 * hl:64 * (hl + 1), q4 * 512:(q4 + 1) * 512],
                                start=True,
                                stop=True,
                                tile_position=(64 * hl, 0),
                            )
                            if masked:
                                nc.vector.tensor_add(
                                    pst, pst,
                                    mt_s[:, j - 4 * q4, :] if mode == "causal" else amt_t,
                                )
                            et_t = etp.tile([128, 512], BF16, tag=f"et{hl}", name=f"et_{hl}")
                            nc.scalar.activation(et_t, pst, Exp)
                            nc.sync.dma_start(
                                out=attnwT[h, j * 128:(j + 1) * 128, qsl],
                                in_=et_t,
                            )
                            nc.tensor.matmul(
                                ot_ps[h],
                                vv[:, j, h, :],
                                et_t,
                                start=(j == 0),
                                stop=(j == nj - 1),
                            )
                    # normalize o columns by the row sums (row 64 of otp)
                    for h in heads:
                        hl = h % 2
                        csl = slice(512 * hl, 512 * hl + 512)
                        srow = small.tile([1, 512], F32, tag=f"srow{hl}", name=f"srow_{hl}")
                        nc.vector.tensor_copy(srow, otp[64:65, csl])
                        nc.vector.reciprocal(srow, srow)
                        nc.scalar.dma_start(out=sums_scr[h, qsl], in_=srow)
                        rb = rbp.tile([64, 512], F32, tag=f"rb{hl}", name=f"rb_{hl}")
                        nc.sync.dma_start(
                            out=rb,
                            in_=bass.AP(
                                tensor=sums_scr.tensor,
                                offset=sums_scr.offset + h * S + q4 * 512,
                                ap=[[0, 64], [1, 512]],
                            ),
                        )
                        if hl == 0:
                            nc.vector.tensor_mul(oT[0:64, pair, qsl], otp[0:64, csl], rb)
                        else:
                            ostg = rbp.tile([64, 512], F32R, tag="ostg", name="ostg")
                            nc.vector.tensor_mul(ostg, otp[0:64, csl], rb)
                            nc.scalar.dma_start(out=oT[64:128, pair, qsl], in_=ostg)
                # ---- output projection for this q4 chunk -> yT ----
                for dm in range(ND8):
                    yp = ps_proj.tile([128, 512], F32, tag="pp")
                    for c2 in range(2):
                        nc.tensor.matmul(
                            yp,
                            wo_s[:, c2, dm * 128:(dm + 1) * 128],
                            oT[:, c2, qsl],
                            start=(c2 == 0),
                            stop=(c2 == 1),
                        )
                    ys = ystg.tile([128, 512], F32, tag="ys")
                    nc.vector.tensor_copy(ys, yp)
                    nc.gpsimd.dma_start(
                        out=yT[dm * 128:(dm + 1) * 128, qsl],
                        in_=ys,
                    )
            # ship the row sums to the host
            nc.scalar.dma_start(out=sums_o[:, :], in_=sums_scr[:, :])

    nc.compile()
    return nc


_PROGRAMS = {}


def _get_program(mode):
    if mode not in _PROGRAMS:
        _PROGRAMS[mode] = _build(mode)
    return _PROGRAMS[mode]


def _causal_mask_tiles():
    p = np.arange(128, dtype=np.int64)[:, None]
    c = np.arange(512, dtype=np.int64)[None, :]
    mt = np.zeros((4, 128, 512), np.float32)
    for m in range(4):
        mt[m] = np.where(c >= 128 * m + p, 0.0, NEG)
    return mt


def _detect_mode(mask):
    tril = np.tril(np.ones((S, S), np.int32))
    if all(np.array_equal(np.asarray(mask[b]), tril) for b in range(B)):
        return "causal"
    if np.all(np.asarray(mask) != 0):
        return "full"
    return "masked"


def make_in_maps(Q, K, V, mask, W_Q, W_K, W_V, W_O, mode):
    mt = _causal_mask_tiles()
    in_maps = []
    for c in range(N_CORES):
        b, g = c // 4, c % 4
        hs = slice(DCORE * g, DCORE * (g + 1))
        m = {
            "xqT": np.ascontiguousarray(Q[b].T).astype(ml_dtypes.bfloat16),
            "xkT": np.ascontiguousarray(K[b].T).astype(ml_dtypes.bfloat16),
            "xvT": np.ascontiguousarray(V[b].T).astype(ml_dtypes.bfloat16),
            "wqT": np.ascontiguousarray((W_Q[hs, :] * 0.125).T).astype(ml_dtypes.bfloat16),
            "wkT": np.ascontiguousarray(W_K[hs, :].T).astype(ml_dtypes.bfloat16),
            "wvT": np.ascontiguousarray(W_V[hs, :].T).astype(ml_dtypes.bfloat16),
            "woT": np.ascontiguousarray(
                np.stack([W_O[:, DCORE * g + 128 * c2: DCORE * g + 128 * (c2 + 1)].T
                          for c2 in range(2)])),
        }
        if mode == "causal":
            m["mask_et"] = mt
        if mode == "masked":
            am = np.where(np.asarray(mask[b]) == 0, np.float32(NEG), np.float32(0.0))
            m["amaskT"] = np.ascontiguousarray(am.T)
        in_maps.append(m)
    return in_maps


def _assemble(results, Q, ln_gamma, ln_beta):
    attn = np.empty((B, NUM_HEADS, S, S), np.float32)
    out = np.empty((B, S, D_MODEL), np.float32)
    for b in range(B):
        y = np.zeros((S, D_MODEL), np.float64)
        for g in range(GROUPS):
            res = results[b * 4 + g]
            rinv = np.asarray(res["sums"], np.float32)
            eT = res["attnwT"]
            for h in range(HPC):
                a = attn[b, 4 * g + h]
                a[...] = np.asarray(eT[h], np.float32).T
                a *= rinv[h][:, None]
            y += res["yT"].T
        y += np.asarray(Q[b], np.float64)
        mu = y.mean(axis=-1, keepdims=True)
        var = ((y - mu) ** 2).mean(axis=-1, keepdims=True)
        o = (y - mu) / np.sqrt(var + LN_EPS)
        o = o * np.asarray(ln_gamma, np.float64) + np.asarray(ln_beta, np.float64)
        out[b] = o.astype(np.float32)
    return out, attn


def kernel(Q, K, V, mask, W_Q, W_K, W_V, W_O, ln_gamma, ln_beta):
    Q = np.asarray(Q, np.float32)
    K = np.asarray(K, np.float32)
    V = np.asarray(V, np.float32)
    W_Q = np.asarray(W_Q, np.float32)
    W_K = np.asarray(W_K, np.float32)
    W_V = np.asarray(W_V, np.float32)
    W_O = np.asarray(W_O, np.float32)
    mode = _detect_mode(mask)
    nc = _get_program(mode)
    in_maps = make_in_maps(Q, K, V, mask, W_Q, W_K, W_V, W_O, mode)
    res = run_bass_kernel_spmd(nc, in_maps, core_ids=list(range(N_CORES)))
    return _assemble(res.results, Q, ln_gamma, ln_beta)
